# revision 38
# baseline (speedup 1.0000x reference)
"""MiniRocket feature extractor on 8 Trainium2 NeuronCores (packed-partition
version).

Per core (4 batch rows). The 6 dilations x 84 kernels = 504 response rows per
batch row are packed into 4 tiles of 126 partitions (engine cost is
proportional to free-dim size only, so 84-row tiles waste 1/3 of every pass):

  T0 = d0 k0..83  | d1 k0..41      T1 = d1 k42..83 | d2 k0..83
  T2 = d3 k0..83  | d4 k0..41      T3 = d4 k42..83 | d5 k0..83

With this packing the global feature row index is simply 126*T + r, so the
output layout is preserved with no permutation.

Per packed tile:
  - two dilations' xshift stacks [72, 4096] (fp16 -> half the DMA bytes of
    fp32; conv error ~1e-3 abs on a resp scale of ~5) feed accumulating
    matmul pairs into one PSUM tile [126, 512]-chunk-wise,
  - ScalarE drains PSUM fp32 -> SBUF int16 with a per-row scale placing all
    four bias thresholds inside +/-32000,
  - the four full-range threshold counts run as fused compare+accumulate
    passes, mostly on VectorE (4x int16 mode, ~0.26ns/elem) with a few moved
    to ScalarE as half-split sign-accumulate passes to balance the two
    engines (Pool has neither compare ALU ops nor a free-axis reduce),
  - ScalarE's long passes are split in 2048-column chunks and deferred two
    tiles so they slot into its idle windows without stalling the PSUM
    ping-pong (a late drain idles PE and drops its pstate ramp),
  - VectorE output-discard tiles rotate 4-wide so consecutive ops have no
    WAW semaphore chain; dependency-free warm-up matmuls anchor the PE
    pstate ramp at t~0,
  - Pool (gpsimd) applies the finalize (feat = full*A - edges*B - C with
    normalization baked in host-side) and the batched per-row output DMAs.
"""

import numpy as np
from contextlib import ExitStack

import concourse.bass as bass
import concourse.mybir as mybir
import concourse.tile as tile
from concourse.ap import AP
from concourse.bass_utils import run_bass_kernel_spmd

DILATIONS = (1, 2, 4, 8, 16, 32)
ND = 6
K = 84
KS = 9
C = 8
L = 4096
F = 4
B = 32
N_CORES = 8
B_LOC = 4
PADMAX = 128
LP = L + 2 * PADMAX
NFEAT = ND * K * F
NP = 126           # packed partition rows per tile
NT = 4             # packed tiles per batch row

# (dil_idx, k_start, count, row_start) pairs per packed tile
GROUPS = (
    ((0, 0, 84, 0), (1, 0, 42, 84)),
    ((1, 42, 42, 0), (2, 0, 84, 42)),
    ((3, 0, 84, 0), (4, 0, 42, 84)),
    ((4, 42, 42, 0), (5, 0, 84, 42)),
)

# Engine for each full-range count pass, keyed (g = b*4 + T, f).
# Default VectorE; 'A' = ScalarE sign-accumulate (finalize coefs adjusted
# host-side per (b, T, f)). Pool's ALU has no compare ops and no reduce
# form, so counting is split between VectorE and ScalarE only.
ENG_FULL = {}
for _g in (2, 6, 10, 15):
    ENG_FULL[(_g, 3)] = "A"
ENG_FULL[(13, 3)] = "S"  # h0 on VectorE, h1 as ScalarE sign

# tiles whose full counts run as two half-range passes (the two halves sum in
# the finalize): used at the pipeline head/tail to cut latency
HALF_CNT = {0, 1, 3, 15}

# cpack column layout
CP_BIAS = 0            # [126, 16]  scaled biases, col = T*4 + f
CP_NBIAS = 16          # [126, 16]  negated scaled biases (ScalarE sign bias)
CP_A = 32              # [126, 64]  full-count coef, col = b*16 + T*4 + f
CP_C = 96              # [126, 64]  constant term, col = b*16 + T*4 + f
CP_B1 = 160            # [126, 16]  edge coef, group-0 rows (zero elsewhere)
CP_B2 = 176            # [126, 16]  edge coef, group-1 rows (zero elsewhere)
CP_SS = 192            # [126, 4]   drain scale, col = T
CP_A2 = 196            # [126, 64]  coef for the h1/aux accumulator column
CP_COLS = 260

F32 = mybir.dt.float32
F16 = mybir.dt.float16
I16 = mybir.dt.int16


def _split_excess_waits(nc, max_waits=1):
    """This walrus build allows only one sync-wait per instruction; hoist
    extra waits onto preceding NOPs of the same engine."""
    n = 0
    for f in nc.m.functions:
        for bb in f.blocks:
            insts = bb.instructions
            if not any(
                i.sync_info and i.sync_info.on_wait and len(i.sync_info.on_wait) > max_waits
                for i in insts
            ):
                continue
            out = []
            for inst in insts:
                si = inst.sync_info
                waits = list(si.on_wait) if si and si.on_wait else []
                if len(waits) > max_waits:
                    for w in waits[:-max_waits]:
                        nop = mybir.InstNoOp(name=f"syncfix-{n}", ins=[], outs=[])
                        n += 1
                        nop.engine = inst.engine
                        nop.sync_info = mybir.SyncInfo(on_wait=[w], on_update=[])
                        out.append(nop)
                    inst.sync_info = mybir.SyncInfo(
                        on_wait=waits[-max_waits:],
                        on_update=list(si.on_update or []),
                    )
                out.append(inst)
            bb.instructions = out


LABELS = {}


def _lab(inst, label):
    LABELS[inst.ins.name] = label
    return inst


def _build_nc():
    nc = bass.Bass()
    xprep = nc.declare_dram_parameter("xprep", [B_LOC, C, LP], F16, isOutput=False)
    wstack = nc.declare_dram_parameter("wstack", [2 * NT, 72, NP], F16, isOutput=False)
    cpack = nc.declare_dram_parameter("cpack", [128, CP_COLS], F32, isOutput=False)
    out = nc.declare_dram_parameter("out", [B_LOC, NFEAT], F32, isOutput=True)

    alu = mybir.AluOpType

    with tile.TileContext(nc) as tc, ExitStack() as ctx:
        cpool = ctx.enter_context(tc.tile_pool(name="const", bufs=1))
        xsh_pool = ctx.enter_context(tc.tile_pool(name="xsh", bufs=8))
        psum_pool = ctx.enter_context(tc.tile_pool(name="psum", bufs=2, space="PSUM"))
        resp_pool = ctx.enter_context(tc.tile_pool(name="resp", bufs=5))
        trash_pool = ctx.enter_context(tc.tile_pool(name="trash", bufs=1))
        acc_pool = ctx.enter_context(tc.tile_pool(name="acc", bufs=8))
        feat_pool = ctx.enter_context(tc.tile_pool(name="feat", bufs=12))

        # weights first (small), then the first two input tiles in half-tile
        # chunks so the first matmuls start as early as possible
        xsh = {}
        order = []
        for b in range(B_LOC):
            for d in range(ND):
                order.append((b, d))

        # bass_priority is an ascending sequence slot (lower = earlier);
        # pin the load order: weights, first two x tiles (in halves so the
        # first matmuls start sooner), constants, then the prefetch stream.
        # critical-path loads spread across three DGE queues so descriptor
        # generation runs in parallel: vector queue carries weights+constants,
        # scalar queue carries the d1 halves, sync queue starts with d0-h0.
        w_t = cpool.tile([72, 2 * NT * NP], F16)

        xt0 = xsh_pool.tile([72, L], F16, name="xt0", tag="t")
        xt1 = xsh_pool.tile([72, L], F16, name="xt1", tag="t")
        xsh[(0, 0)] = xt0
        xsh[(0, 1)] = xt1
        def _xpart(ti, d, c0, c1, queue, prio):
            _i = queue.dma_start(
                ti[:, c0:c1],
                AP(xprep, PADMAX - 4 * d + c0, [[d, KS], [LP, C], [1, c1 - c0]]))
            _i.ins.bass_priority = prio

        _iw0 = nc.scalar.dma_start(w_t[:, 0 : 2 * NP], AP(wstack, 0, [[NP, 72], [72 * NP, 2], [1, NP]]))
        _iw0.ins.bass_priority = 1
        _xpart(xt0, DILATIONS[0], 0, 2048, nc.sync, 2)
        _xpart(xt1, DILATIONS[1], 0, 2048, nc.gpsimd, 3)
        cp_t = cpool.tile([128, CP_COLS], F32, tag="cp_t")
        _ic = nc.scalar.dma_start(cp_t[:], AP(cpack, 0, [[CP_COLS, 128], [1, CP_COLS]]))
        _ic.ins.bass_priority = 4
        _xpart(xt0, DILATIONS[0], 2048, 4096, nc.sync, 5)
        _xpart(xt1, DILATIONS[1], 2048, 4096, nc.gpsimd, 6)
        _iw1 = nc.scalar.dma_start(
            w_t[:, 2 * NP :],
            AP(wstack, 2 * 72 * NP, [[NP, 72], [72 * NP, 2 * NT - 2], [1, NP]]))
        _iw1.ins.bass_priority = 7
        _prio = 8

        # remaining input loads, in first-use order; bufs=8 turns this into a
        # sliding prefetch window on the in-order sync queue
        for (b, d) in order[2:]:
            t = xsh_pool.tile([72, L], F16)
            _x = _lab(nc.sync.dma_start(
                t[:],
                AP(xprep, b * C * LP + (PADMAX - 4 * DILATIONS[d]), [[DILATIONS[d], KS], [LP, C], [1, L]]),
            ), f"xdma b{b} d{d}")
            _x.ins.bass_priority = _prio
            _prio += 1
            xsh[(b, d)] = t

        # rotate discard-output tiles so consecutive DVE ops have no
        # write-after-write semaphore chain (costs ~95ns/op otherwise)
        trash = [trash_pool.tile([NP, L], I16, name=f"trash{i}", tag=f"trash{i}") for i in range(4)]
        trash_a = [trash_pool.tile([NP, L], I16, name=f"trash_a{i}", tag=f"trash_a{i}") for i in range(2)]
        _tri = [0]

        def _trash():
            t = trash[_tri[0] % 4]
            _tri[0] += 1
            return t

        # PE pstate warm-up: dependency-free dummy matmuls into the first
        # PSUM tile anchor the tensor engine's ramp clock at t~0 so the real
        # matmuls run at full rate. Results are overwritten by the first
        # real start=True matmul.
        warm_w = cpool.tile([72, 640], F16, tag="warm_w")
        nc.gpsimd.memset(warm_w[:], 0.0)

        featb_of = {}
        pending = {}

        def _flush(gq):
            """Emit the deferred ScalarE sign counts, finalize and output for
            tile gq (two tiles after its drains, so ScalarE's long sign passes
            land in its idle window instead of delaying the drain pipeline)."""
            info = pending.pop(gq)
            bq, Tq = info["b"], info["T"]
            racc, rresp = info["acc"], info["resp"]
            for f in range(4):
                if ENG_FULL.get((gq, f)) == "S":
                    col = Tq * 4 + f
                    _lab(nc.scalar.activation(
                        trash_a[1][:, 2048:4096],
                        rresp[:, 2048:4096],
                        mybir.ActivationFunctionType.Sign,
                        bias=cp_t[0:NP, CP_NBIAS + col : CP_NBIAS + col + 1],
                        accum_out=racc[:, 3 * F + f : 3 * F + f + 1],
                    ), f"signA g{gq} f{f} h1")
                if ENG_FULL.get((gq, f)) == "A":
                    col = Tq * 4 + f
                    # half-split so ScalarE's longest contiguous block stays
                    # under the PSUM-recycle tolerance (~3.4us)
                    for h in range(2):
                        ac = f if h == 0 else 3 * F + f
                        _lab(nc.scalar.activation(
                            trash_a[h][:, h * 2048 : (h + 1) * 2048],
                            rresp[:, h * 2048 : (h + 1) * 2048],
                            mybir.ActivationFunctionType.Sign,
                            bias=cp_t[0:NP, CP_NBIAS + col : CP_NBIAS + col + 1],
                            accum_out=racc[:, ac : ac + 1],
                        ), f"signA g{gq} f{f} h{h}")
            ca = bq * 16 + Tq * 4
            featb = featb_of[bq]
            eng = nc.vector if gq == NT * B_LOC - 1 else nc.gpsimd
            u = feat_pool.tile([NP, F], F32)
            _lab(eng.tensor_mul(u[:], racc[:, 0:F], cp_t[0:NP, CP_A + ca : CP_A + ca + F]), f"fin g{gq} u")
            w2 = feat_pool.tile([NP, F], F32)
            _lab(eng.tensor_mul(w2[:], racc[:, F : 2 * F], cp_t[0:NP, CP_B1 + Tq * 4 : CP_B1 + Tq * 4 + F]), f"fin g{gq} w2")
            w3 = feat_pool.tile([NP, F], F32)
            _lab(eng.tensor_mul(w3[:], racc[:, 2 * F : 3 * F], cp_t[0:NP, CP_B2 + Tq * 4 : CP_B2 + Tq * 4 + F]), f"fin g{gq} w3")
            u2 = feat_pool.tile([NP, F], F32)
            _lab(eng.tensor_mul(u2[:], racc[:, 3 * F : 4 * F], cp_t[0:NP, CP_A2 + ca : CP_A2 + ca + F]), f"fin g{gq} u2")
            ua = feat_pool.tile([NP, F], F32)
            _lab(eng.tensor_add(ua[:], u[:], u2[:]), f"fin g{gq} ua")
            u = ua
            ft = feat_pool.tile([NP, F], F32)
            _lab(eng.tensor_sub(ft[:], u[:], w2[:]), f"fin g{gq} ft")
            ft2 = feat_pool.tile([NP, F], F32)
            _lab(eng.tensor_sub(ft2[:], ft[:], w3[:]), f"fin g{gq} ft2")
            _lab(eng.tensor_sub(
                featb[:, Tq * F : (Tq + 1) * F], ft2[:],
                cp_t[0:NP, CP_C + ca : CP_C + ca + F],
            ), f"fin g{gq} out")
            last_b = B_LOC - 1
            if bq == last_b and Tq == NT - 2:
                # ship the first three quarters of the last batch row early
                dst = AP(out, bq * NFEAT, [[F, NP], [NP * F, NT - 1], [1, F]])
                _lab(nc.gpsimd.dma_start(dst, featb[:, 0 : (NT - 1) * F]), f"outdma b{bq}p0")
            elif bq == last_b and Tq == NT - 1:
                dst = AP(out, bq * NFEAT + (NT - 1) * NP * F, [[F, NP], [1, F]])
                _lab(nc.gpsimd.dma_start(dst, featb[:, (NT - 1) * F :]), f"outdma b{bq}p1")
            elif Tq == NT - 1:
                dst = AP(out, bq * NFEAT, [[F, NP], [NP * F, NT], [1, F]])
                _lab(nc.gpsimd.dma_start(dst, featb[:]), f"outdma b{bq}")

        for b in range(B_LOC):
            featb_of[b] = feat_pool.tile([NP, NT * F], F32, name=f"featb{b}", tag=f"featb{b}")
            for T in range(NT):
                g = b * NT + T
                (dA, _, _, _), (dB, _, _, _) = GROUPS[T]
                xA = xsh[(b, dA)]
                xB = xsh[(b, dB)]
                wA = w_t[:, (2 * T) * NP : (2 * T + 1) * NP]
                wB = w_t[:, (2 * T + 1) * NP : (2 * T + 2) * NP]

                resp = resp_pool.tile([NP, L], I16)
                for h in range(2):
                    ps = psum_pool.tile([NP, 2048], F32)
                    if g == 0 and h == 0:
                        # PE pstate warm-up: dependency-free dummy matmuls
                        # anchor the ramp clock at t~0; overwritten by the
                        # first real start=True matmul below.
                        for wi in range(4):
                            _lab(nc.tensor.matmul(
                                ps[:, 0:512], warm_w[:, 0:NP], warm_w[:, 128:640],
                                start=True, stop=True, skip_group_check=True,
                            ), f"warm mm{wi}")
                    for n in range(4):
                        c0 = h * 2048 + n * 512
                        _lab(nc.tensor.matmul(
                            ps[:, n * 512 : (n + 1) * 512], wA, xA[:, c0 : c0 + 512],
                            start=True, stop=False,
                        ), f"mmA g{g} h{h} n{n}")
                    for n in range(4):
                        c0 = h * 2048 + n * 512
                        _lab(nc.tensor.matmul(
                            ps[:, n * 512 : (n + 1) * 512], wB, xB[:, c0 : c0 + 512],
                            start=False, stop=True,
                        ), f"mmB g{g} h{h} n{n}")
                    _lab(nc.scalar.activation(
                        resp[:, h * 2048 : (h + 1) * 2048], ps[:],
                        mybir.ActivationFunctionType.Copy,
                        scale=cp_t[0:NP, CP_SS + T : CP_SS + T + 1],
                    ), f"drainA g{g} h{h}")

                acc = acc_pool.tile([NP, 4 * F], F32)
                nc.gpsimd.memset(acc[:, 3 * F : 4 * F], 0.0)
                for f in range(4):
                    if ENG_FULL.get((g, f)) == "A":
                        continue
                    col = T * 4 + f
                    if ENG_FULL.get((g, f)) == "S":
                        _lab(nc.vector.tensor_scalar(
                            _trash()[:, 0:2048], resp[:, 0:2048],
                            cp_t[0:NP, CP_BIAS + col : CP_BIAS + col + 1], None,
                            alu.is_gt, alu.add,
                            accum_out=acc[:, f : f + 1],
                        ), f"cntV g{g} f{f} h0")
                        continue
                    if g in HALF_CNT:
                        for h in range(2):
                            ac = f if h == 0 else 3 * F + f
                            _lab(nc.vector.tensor_scalar(
                                _trash()[:, h * 2048 : (h + 1) * 2048],
                                resp[:, h * 2048 : (h + 1) * 2048],
                                cp_t[0:NP, CP_BIAS + col : CP_BIAS + col + 1], None,
                                alu.is_gt, alu.add,
                                accum_out=acc[:, ac : ac + 1],
                            ), f"cntV g{g} f{f} h{h}")
                    else:
                        _lab(nc.vector.tensor_scalar(
                            _trash()[:], resp[:],
                            cp_t[0:NP, CP_BIAS + col : CP_BIAS + col + 1], None,
                            alu.is_gt, alu.add,
                            accum_out=acc[:, f : f + 1],
                        ), f"cntV g{g} f{f}")

                # pad-edge counts: both edges in one strided op per (group, f).
                # Engine partition ranges must start at 0, so each group's op
                # covers all 126 rows into its own acc columns; the finalize
                # coefs (B1/B2) zero out the wrong-pad rows.
                pstep = resp[:].ap[0][0]
                for gi, (dg, _, _, _) in enumerate(GROUPS[T]):
                    pad = 4 * DILATIONS[dg]
                    for f in range(4):
                        col = T * 4 + f
                        tv = _trash()
                        tstep = tv[:].ap[0][0]
                        ein = AP(
                            resp[:].tensor, resp[:].offset,
                            [[pstep, NP], [L - pad, 2], [1, pad]],
                        )
                        eout = AP(
                            tv[:].tensor, tv[:].offset,
                            [[tstep, NP], [L - pad, 2], [1, pad]],
                        )
                        _lab(nc.vector.tensor_scalar(
                            eout, ein,
                            cp_t[0:NP, CP_BIAS + col : CP_BIAS + col + 1], None,
                            alu.is_gt, alu.add,
                            accum_out=acc[:, (1 + gi) * F + f : (1 + gi) * F + f + 1],
                        ), f"edge g{g} gi{gi} f{f}")

                pending[g] = {"b": b, "T": T, "resp": resp, "acc": acc}
                if g - 2 in pending:
                    _flush(g - 2)

        _flush(NT * B_LOC - 2)
        _flush(NT * B_LOC - 1)

    _split_excess_waits(nc)
    return nc


_NC_CACHE = None


def _get_nc():
    global _NC_CACHE
    if _NC_CACHE is None:
        _NC_CACHE = _build_nc()
    return _NC_CACHE


LAST_RESULTS = None


def kernel(x, channel_masks, bias_matrices, feature_mean, feature_std):
    global LAST_RESULTS
    x = np.ascontiguousarray(np.asarray(x, dtype=np.float32))
    masks = np.asarray(channel_masks, dtype=np.float32)
    biasm = np.asarray(bias_matrices, dtype=np.float32)
    mean = np.asarray(feature_mean, dtype=np.float32)
    std = np.asarray(feature_std, dtype=np.float32)

    # packed weight stacks: one [72, 126] matrix per (tile, group)
    wpack = np.zeros((2 * NT, 72, NP), np.float16)
    for T in range(NT):
        for gi, (d, k0, cnt, r0) in enumerate(GROUPS[T]):
            wt = -masks[d, k0 : k0 + cnt].T  # [C, cnt]
            for j in range(KS):
                wpack[2 * T + gi, j * C : (j + 1) * C, r0 : r0 + cnt] = wt

    # int16 drain: resp stored as round(resp * s_dk); compare against
    # bias * s_dk. s_dk chosen so all four thresholds sit inside +/-32000.
    maxb = np.maximum(np.abs(biasm).max(axis=-1), 1e-6)  # [ND, K]
    sscale = 32000.0 / maxb                              # [ND, K]
    bias_s = biasm * sscale[:, :, None]                  # [ND, K, F]

    # normalization-baked finalize coefs per (d, k, f)
    coef_a = np.zeros((ND, K, F), np.float32)
    coef_b = np.zeros((ND, K, F), np.float32)
    coef_c = np.zeros((ND, K, F), np.float32)
    meanr = mean.reshape(ND, K, F)
    stdr = std.reshape(ND, K, F)
    for di, d in enumerate(DILATIONS):
        pad = 4 * d
        lt = L - 2 * pad
        par = ((di + np.arange(K)) % 2 == 1).astype(np.float32)[:, None]
        A = np.where(par > 0, 1.0 / lt, 1.0 / L)
        Bc = np.where(par > 0, 1.0 / lt, 0.0)
        coef_a[di] = A / stdr[di]
        coef_b[di] = Bc / stdr[di]
        coef_c[di] = meanr[di] / stdr[di]

    # packed per-row tables, row r of tile T = (d, k) per GROUPS
    bias_pk = np.zeros((NP, NT, F), np.float32)
    ss_pk = np.ones((NP, NT), np.float32)
    a_pk = np.zeros((NP, NT, F), np.float32)
    b1_pk = np.zeros((NP, NT, F), np.float32)
    b2_pk = np.zeros((NP, NT, F), np.float32)
    c_pk = np.zeros((NP, NT, F), np.float32)
    for T in range(NT):
        for gi, (d, k0, cnt, r0) in enumerate(GROUPS[T]):
            sl = slice(r0, r0 + cnt)
            ks = slice(k0, k0 + cnt)
            bias_pk[sl, T] = bias_s[d, ks]
            ss_pk[sl, T] = sscale[d, ks]
            a_pk[sl, T] = coef_a[d, ks]
            (b1_pk if gi == 0 else b2_pk)[sl, T] = coef_b[d, ks]
            c_pk[sl, T] = coef_c[d, ks]

    # per-(b, T, f) A/C (ScalarE sign-counted passes accumulate
    # sum(sign(resp-b)) = 2*count - L: halve A, shift C)
    a_btf = np.broadcast_to(a_pk[:, None], (NP, B_LOC, NT, F)).copy()
    c_btf = np.broadcast_to(c_pk[:, None], (NP, B_LOC, NT, F)).copy()
    a2_btf = a_btf.copy()
    for (g, f), eng in ENG_FULL.items():
        b_, T_ = divmod(g, NT)
        if eng == "A":
            # both halves are sign-sums S_h = 2c_h - L/2
            a_btf[:, b_, T_, f] *= 0.5
            a2_btf[:, b_, T_, f] = a_btf[:, b_, T_, f]
            c_btf[:, b_, T_, f] -= float(L) * a_btf[:, b_, T_, f]
        elif eng == "S":
            # h0 is a plain count, h1 is a sign-sum S = 2c_h1 - L/2
            a2_btf[:, b_, T_, f] = 0.5 * a_btf[:, b_, T_, f]
            c_btf[:, b_, T_, f] -= (float(L) / 2) * a2_btf[:, b_, T_, f]

    cpk = np.zeros((128, CP_COLS), np.float32)
    cpk[:NP, CP_BIAS : CP_BIAS + 16] = bias_pk.reshape(NP, 16)
    cpk[:NP, CP_NBIAS : CP_NBIAS + 16] = -bias_pk.reshape(NP, 16)
    cpk[:NP, CP_A : CP_A + 64] = a_btf.reshape(NP, 64)
    cpk[:NP, CP_C : CP_C + 64] = c_btf.reshape(NP, 64)
    cpk[:NP, CP_B1 : CP_B1 + 16] = b1_pk.reshape(NP, 16)
    cpk[:NP, CP_B2 : CP_B2 + 16] = b2_pk.reshape(NP, 16)
    cpk[:NP, CP_SS : CP_SS + 4] = ss_pk
    cpk[:NP, CP_A2 : CP_A2 + 64] = a2_btf.reshape(NP, 64)

    xt = np.ascontiguousarray(x.transpose(0, 2, 1))
    xp = np.zeros((B, C, LP), np.float16)
    xp[:, :, PADMAX : PADMAX + L] = xt.astype(np.float16)

    nc = _get_nc()
    in_maps = []
    for core in range(N_CORES):
        in_maps.append(
            {
                "xprep": np.ascontiguousarray(xp[core * B_LOC : (core + 1) * B_LOC]),
                "wstack": wpack,
                "cpack": cpk,
            }
        )
    res = run_bass_kernel_spmd(nc, in_maps, list(range(N_CORES)))
    LAST_RESULTS = res
    out = np.concatenate([res.results[i]["out"] for i in range(N_CORES)], axis=0)
    return out.astype(np.float32)
